# revision 31
# baseline (speedup 1.0000x reference)
"""Trainium2 Bass kernel for nn_BaseModel_75522704933527 (gnn_message_passing).

Math (exactly equivalent to the reference; everything else is dead code since
the head only reads feats[0][:,0,:], _cg_iterate is per-l independent, and
l=0 has no U2 coupling):

    d      = |pos[n] - pos[c] + (shift-1) @ cells[sp]|            per pair
    Rk0    = radialMLP(d)[:, :128]    (rad weights)
    Rke0   = radialMLP(d)[:, :128]    (erad weights)
    feats0 = segsum_c(IS * Rk0 * embed[species[n]]) * MS          [A, 128]
    feats0 += mix_a[0] * feats0**2
    new0   = feats0 + segsum_c((1+IS) * Rke0 * feats0[n]) * MS
    new0   += emix_a[0] * new0**2
    out    = MLP_head(new0)                                       [A, 1]

Sharding: atoms split 625/core across 8 cores; each core owns all pairs whose
center is in its range (segment sums need no cross-core reduction). Pairs are
sorted by center and padded so each 128-atom block owns exactly TB 128-pair
tiles (uniform -> one SPMD program). Scatter = PE matmul with an is_equal
selection matrix accumulated in PSUM per block. feats0 is AllGather'd (bf16)
mid-kernel for the layer-2 neighbor gather.

v2 (instruction-count optimized):
  - species embedding folded into the radial matmul: lhsT rows 65:69 hold the
    one-hot species rows, rhs cols 256:384 hold embed*IS*MS -> no per-tile
    embedding gather.
  - layer-2 gather: ONE batched indirect DMA per block (SWDGE fixed cost
    ~1us amortized over TB*128 descriptors) from bf16 feats0_full.
  - s01 selection matrices built for a whole block in one DVE op
    (broadcast APs, all-bf16 for fast DVE modes).
  - rt matmuls write [P,384] f32 PSUM supertiles (2 tiles / 2 banks); msg and
    the Rke stash are strided batched ops over the supertile.
  - msg2 for a whole block in one all-bf16 DVE op.
  - silu via ACTF.Silu (one op instead of sigmoid+mul).
"""
import numpy as np
import ml_dtypes

import concourse.bass as bass
import concourse.mybir as mybir
import concourse.tile as tile
from concourse import bacc
from concourse.bass_utils import run_bass_kernel_spmd
from concourse.masks import make_identity

F32 = mybir.dt.float32
BF16 = mybir.dt.bfloat16
I32 = mybir.dt.int32
I16 = mybir.dt.int16
ALU = mybir.AluOpType
ACTF = mybir.ActivationFunctionType

BF16NP = ml_dtypes.bfloat16

NCORES = 8
N_ATOMS = 5000
K = 128
NB = 8           # radial basis size
NH = 32          # radial MLP hidden per net (rad + erad stacked -> 64)
CUTOFF = 5.0
MSG_SCALE = 0.1767767
INIT_SCALE = 0.2
P = 128
NBLK = 5                        # atom blocks per core
AC = NBLK * P                   # 640 atom slots per core (128-aligned)
CAT = 2 * NH + 1                # 65 rows: rad-hidden|erad-hidden|ones

_prog_cache = {}


def _build_program(TB, debug=False):
    """Build the SPMD bass program for per-(core,block) tile count TB."""
    T = NBLK * TB                # seg tiles per core
    TBP = TB * P                 # padded pairs per block
    G2 = (TB + 1) // 2           # 2-tile supertile groups per block

    nc = bacc.Bacc(None, target_bir_lowering=False,
                   num_swdge_queues=4)

    def din(name, shape, dt=F32):
        return nc.dram_tensor(name, shape, dt, kind="ExternalInput")

    plane_names = (['pcx', 'pcy', 'pcz', 'pnx', 'pny', 'pnz',
                    'sh0', 'sh1', 'sh2'] +
                   [f'cl{i}' for i in range(9)])
    planes_d = din('planes', [len(plane_names), P, T])
    T16 = NBLK * (TBP // 16)     # wrapped int16 index columns
    sel_d = din('sel', [P, T * P], BF16)    # host-built is_equal planes
    ones_d = din('ones', [1, TBP], BF16)
    nbr16_d = din('nbr16', [P, T16], I16)
    embp_d = din('embp', [P, T * K], BF16)   # IS*MS*embed[spc[nbr]] per slot
    w1cat9_d = din('w1cat9', [NB + 1, 2 * NH], BF16)   # row 8 = [b1|eb1]
    w2e_d = din('w2e', [CAT, 2 * K], BF16)
    mix0m_d = din('mix0m', [P, K])
    emix0m_d = din('emix0m', [P, K])
    w1h_d = din('w1h', [K, K])
    w2h_d = din('w2h', [K, K])
    wlast_d = din('wlast', [K, 1])
    b1hc_d = din('b1hc', [K, 1])
    b2hc_d = din('b2hc', [K, 1])
    lastb_d = din('lastb', [1, 1])

    out_d = nc.dram_tensor('out', [1, NBLK * P], F32, kind="ExternalOutput")
    if debug:
        dbg_dpl = nc.dram_tensor('dbg_dpl', [P, T], F32, kind="ExternalOutput")
        dbg_cat = nc.dram_tensor('dbg_cat', [CAT, TBP], F32,
                                 kind="ExternalOutput")
        dbg_rt = nc.dram_tensor('dbg_rt', [P, 2 * K], F32,
                                kind="ExternalOutput")
        dbg_emb = nc.dram_tensor('dbg_emb', [P, K], F32,
                                 kind="ExternalOutput")
        dbg_msg = nc.dram_tensor('dbg_msg', [P, K], F32,
                                 kind="ExternalOutput")
        dbg_s01 = nc.dram_tensor('dbg_s01', [P, P], F32,
                                 kind="ExternalOutput")
        dbg_f0 = nc.dram_tensor('dbg_f0', [P, NBLK * K], F32,
                                kind="ExternalOutput")
        dbg_gin = nc.dram_tensor('dbg_gin', [P, K], F32,
                                 kind="ExternalOutput")
        dbg_h0 = nc.dram_tensor('dbg_h0', [P, NBLK * K], F32,
                                kind="ExternalOutput")

    with tile.TileContext(nc) as tc:
        with (
            tc.tile_pool(name="cst", bufs=1) as cst,
            tc.tile_pool(name="geo", bufs=1) as geo,
            tc.tile_pool(name="big", bufs=1) as big,
            tc.tile_pool(name="blk", bufs=2) as blk,
            tc.tile_pool(name="sg", bufs=2) as sg,
            tc.tile_pool(name="ps_w1", bufs=2, space="PSUM") as ps_w1,
            tc.tile_pool(name="ps_rt", bufs=4, space="PSUM") as ps_rt,
            tc.tile_pool(name="ps_acc", bufs=2, space="PSUM") as ps_acc,
            tc.tile_pool(name="dram", bufs=1, space="DRAM") as dram,
        ):
            # ---------------- constants ----------------
            def constcol(v, _cache={}):
                if v not in _cache:
                    t = cst.tile([P, 1], F32, tag=f"cc{len(_cache)}")
                    nc.vector.memset(t[:], float(v))
                    _cache[v] = t
                return _cache[v][:]

            ident = cst.tile([P, P], F32)
            make_identity(nc, ident[:])

            def load_const(dram_t, shape, dt=F32, tag=None):
                t = cst.tile(shape, dt, tag=tag or dram_t.name + "_s")
                nc.sync.dma_start(t[:], dram_t[:])
                return t

            w1cat9 = load_const(w1cat9_d, [NB + 1, 2 * NH], BF16)
            w2e = load_const(w2e_d, [CAT, 2 * K], BF16)
            mix0m = load_const(mix0m_d, [P, K])
            emix0m = load_const(emix0m_d, [P, K])
            w1h = load_const(w1h_d, [K, K])
            w2h = load_const(w2h_d, [K, K])
            wlast = load_const(wlast_d, [K, 1])
            b1hc = load_const(b1hc_d, [K, 1])
            b2hc = load_const(b2hc_d, [K, 1])
            lastb = load_const(lastb_d, [1, 1])

            nbr16 = load_const(nbr16_d, [P, T16], I16)

            planes_t = cst.tile([P, len(plane_names) * T], F32,
                                tag="planes_s")
            nc.sync.dma_start(
                planes_t[:].rearrange("q (n t) -> q n t",
                                      n=len(plane_names), t=T),
                planes_d[:].rearrange("n q t -> q n t"))
            planes = {n: planes_t[:, i * T:(i + 1) * T]
                      for i, n in enumerate(plane_names)}

            # ---------------- stage A: geometry -> d ----------------
            vcomp = []
            for j in range(3):
                v = geo.tile([P, T], F32, tag=f"v{j}")
                nc.vector.tensor_sub(v[:], planes[f'pn{"xyz"[j]}'],
                                     planes[f'pc{"xyz"[j]}'])
                for i in range(3):
                    t = geo.tile([P, T], F32, tag="gt")
                    # (shift_i - 1) * cell[i,j]
                    nc.vector.scalar_tensor_tensor(
                        out=t[:], in0=planes[f'sh{i}'], scalar=1.0,
                        in1=planes[f'cl{3 * i + j}'],
                        op0=ALU.subtract, op1=ALU.mult)
                    nc.vector.tensor_add(v[:], v[:], t[:])
                vcomp.append(v)
            d2 = geo.tile([P, T], F32, tag="d2")
            nc.vector.tensor_mul(d2[:], vcomp[0][:], vcomp[0][:])
            for j in (1, 2):
                t = geo.tile([P, T], F32, tag="gt")
                nc.vector.tensor_mul(t[:], vcomp[j][:], vcomp[j][:])
                nc.vector.tensor_add(d2[:], d2[:], t[:])
            dpl = geo.tile([P, T], F32, tag="dpl")
            nc.scalar.activation(dpl[:], d2[:], ACTF.Sqrt,
                                 bias=constcol(1e-12), scale=1.0)
            if debug:
                nc.sync.dma_start(dbg_dpl[:], dpl[:])

            # ---------------- stage B: radial basis ----------------
            dmin = geo.tile([P, T], F32, tag="dmin")
            nc.vector.tensor_scalar_min(dmin[:], dpl[:], CUTOFF)
            fsin = geo.tile([P, T], F32, tag="fsin")
            nc.scalar.activation(fsin[:], dmin[:], ACTF.Sin,
                                 bias=constcol(np.pi / 2),
                                 scale=float(-np.pi / CUTOFF))
            fcut = geo.tile([P, T], F32, tag="fcut")
            nc.scalar.activation(fcut[:], fsin[:], ACTF.Copy,
                                 bias=0.5, scale=0.5)

            rbf = big.tile([P, NB, T], BF16, tag="rbf")
            centers = np.linspace(0.0, CUTOFF, NB)
            for b in range(NB):
                t = geo.tile([P, T], F32, tag="gt")
                nc.scalar.activation(t[:], dpl[:], ACTF.Square,
                                     bias=constcol(float(-centers[b])),
                                     scale=1.0)
                e = geo.tile([P, T], F32, tag="ge")
                nc.scalar.activation(e[:], t[:], ACTF.Exp,
                                     bias=constcol(0.0), scale=-2.0)
                nc.vector.tensor_mul(rbf[:, b, :], e[:], fcut[:])

            # ---------------- layer 1 ----------------
            feats0 = big.tile([P, NBLK * K], F32, tag="feats0")
            feats0b = big.tile([P, NBLK * K], BF16, tag="feats0b")
            rkes = big.tile([P, T * K], BF16, tag="rkes")
            CH = 512
            feats0_full = dram.tile([NBLK * NCORES * P, K], BF16)
            in_ccs = [dram.tile([P, K], BF16, name=f"incc{b}")
                      for b in range(NBLK)]
            rbf8s, cat69s = [], []
            for v in range(2):
                rb = big.tile([NB + 1, TBP], BF16, tag=f"rbf8{v}",
                              name=f"rbf8{v}")
                nc.sync.dma_start(rb[NB:NB + 1, :], ones_d[:])
                rbf8s.append(rb)
                ct = big.tile([CAT, TBP], BF16, tag=f"cat69{v}",
                              name=f"cat69{v}")
                nc.sync.dma_start(ct[2 * NH:2 * NH + 1, :], ones_d[:])
                cat69s.append(ct)
            for b in range(NBLK):
                # reshuffle block's rbf planes to [9, TBP] via DRAM bounce
                bounce = dram.tile([NB, TBP], BF16, tag="bounce", bufs=2,
                                   name=f"bounce{b}")
                nc.sync.dma_start(
                    bounce[:].rearrange("b (q t) -> q b t", q=P, t=TB),
                    rbf[:, :, b * TB:(b + 1) * TB])
                rbf8 = rbf8s[b % 2]
                nc.sync.dma_start(rbf8[0:NB, :], bounce[:])

                cat69 = cat69s[b % 2]
                embs = blk.tile([P, TB, K], BF16, tag="embs", bufs=2)
                nc.sync.dma_start(
                    embs[:],
                    embp_d[:, b * TB * K:(b + 1) * TB * K]
                    .rearrange("q (j k) -> q j k", j=TB, k=K))
                for c in range((TBP + CH - 1) // CH):
                    lo = c * CH
                    n = min(CH, TBP - lo)
                    hps = ps_w1.tile([2 * NH, CH], F32, tag="hps")
                    nc.tensor.matmul(hps[:, :n], lhsT=w1cat9[:],
                                     rhs=rbf8[:, lo:lo + n],
                                     start=True, stop=True)
                    sgm = sg.tile([2 * NH, CH], BF16, tag="sgm")
                    nc.scalar.activation(sgm[:, :n], hps[:, :n], ACTF.Sigmoid,
                                         bias=constcol(0.0)[:2 * NH],
                                         scale=1.0)
                    nc.vector.tensor_mul(cat69[0:2 * NH, lo:lo + n],
                                         sgm[:, :n], hps[:, :n])

                s01b = blk.tile([P, TB, P], BF16, tag="s01b")
                nc.scalar.dma_start(
                    s01b[:],
                    sel_d[:, b * TBP:(b + 1) * TBP]
                    .rearrange("q (j a) -> q j a", j=TB, a=P))

                f0ps = ps_acc.tile([P, K], F32, tag="facc")
                for g in range(G2):
                    j0 = 2 * g
                    w = min(2, TB - j0)
                    rt = ps_rt.tile([P, 2, 2 * K], F32, tag="rt")
                    for j2 in range(w):
                        j = j0 + j2
                        nc.tensor.matmul(rt[:, j2, :],
                                         lhsT=cat69[:, j * P:(j + 1) * P],
                                         rhs=w2e[:],
                                         start=True, stop=True)
                    msgb = sg.tile([P, 2, K], BF16, tag="msgb", bufs=4)
                    nc.vector.tensor_tensor(out=msgb[:, 0:w, :],
                                            in0=rt[:, 0:w, 0:K],
                                            in1=embs[:, j0:j0 + w, :],
                                            op=ALU.mult)
                    i0 = b * TB + j0
                    nc.vector.tensor_copy(
                        rkes[:, i0 * K:(i0 + w) * K]
                            .rearrange("q (w k) -> q w k", w=w),
                        rt[:, 0:w, K:2 * K])
                    for j2 in range(w):
                        j = j0 + j2
                        nc.tensor.matmul(f0ps[:], lhsT=s01b[:, j, :],
                                         rhs=msgb[:, j2, :],
                                         start=(j == 0), stop=(j == TB - 1))
                # CG-1 (l=0): feats0 = F0 + mix0 * F0^2 = F0*(1 + mix0*F0)
                f0s = sg.tile([P, K], F32, tag="f0s")
                nc.scalar.copy(f0s[:], f0ps[:])
                tmix = sg.tile([P, K], F32, tag="tmix")
                nc.vector.tensor_mul(tmix[:], f0s[:], mix0m[:])
                nc.vector.scalar_tensor_tensor(
                    out=feats0[:, b * K:(b + 1) * K], in0=tmix[:], scalar=1.0,
                    in1=f0s[:], op0=ALU.add, op1=ALU.mult)
                nc.vector.tensor_copy(feats0b[:, b * K:(b + 1) * K],
                                      feats0[:, b * K:(b + 1) * K])
                nc.sync.dma_start(in_ccs[b][:],
                                  feats0b[:, b * K:(b + 1) * K])
                nc.gpsimd.collective_compute(
                    "AllGather", ALU.bypass,
                    replica_groups=[list(range(NCORES))],
                    ins=[in_ccs[b].opt()],
                    outs=[feats0_full[b * NCORES * P:(b + 1) * NCORES * P, :]])
            if debug:
                nc.sync.dma_start(dbg_f0[:], feats0[:])



            # ---------------- layer 2 ----------------
            h0 = big.tile([P, NBLK * K], F32, tag="h0")
            qn = 0
            for b in range(NBLK):
                ginv = blk.tile([P, TB, K], BF16, tag="ginv", bufs=4)
                # chunked (1024-desc carveout) gathers, round-robin queues
                for j0 in range(0, TB, 8):
                    jw = min(8, TB - j0)
                    nc.gpsimd.dma_gather(
                        out_ap=ginv[:, j0:j0 + jw, :], in_ap=feats0_full[:],
                        idxs_ap=nbr16[:, b * TB * 8 + j0 * 8:
                                      b * TB * 8 + (j0 + jw) * 8],
                        num_idxs=jw * P, num_idxs_reg=jw * P, elem_size=K,
                        queue_num=qn % 4)
                    qn += 1
                s01b = blk.tile([P, TB, P], BF16, tag="s01b", name="s01b2")
                nc.scalar.dma_start(
                    s01b[:],
                    sel_d[:, b * TBP:(b + 1) * TBP]
                    .rearrange("q (j a) -> q j a", j=TB, a=P))
                if debug and b == 0:
                    gc = sg.tile([P, K], F32, tag="gc")
                    nc.vector.tensor_copy(gc[:], ginv[:, 0, :])
                    nc.sync.dma_start(dbg_gin[:], gc[:])
                # msg2 = rkes * ginv, in place into the rkes slice
                msg2 = rkes[:, b * TB * K:(b + 1) * TB * K]
                nc.vector.tensor_tensor(
                    out=msg2,
                    in0=msg2,
                    in1=ginv[:].rearrange("q j k -> q (j k)"),
                    op=ALU.mult)
                f1ps = ps_acc.tile([P, K], F32, tag="facc")
                for j in range(TB):
                    nc.tensor.matmul(f1ps[:], lhsT=s01b[:, j, :],
                                     rhs=msg2[:, j * K:(j + 1) * K],
                                     start=(j == 0), stop=(j == TB - 1))
                # new0 = feats0 + F1 ; h0 = new0*(1 + emix0*new0)
                nn = sg.tile([P, K], F32, tag="cgn")
                nc.vector.tensor_add(nn[:], f1ps[:],
                                     feats0[:, b * K:(b + 1) * K])
                tmix = sg.tile([P, K], F32, tag="tmix2")
                nc.vector.tensor_mul(tmix[:], nn[:], emix0m[:])
                nc.vector.scalar_tensor_tensor(
                    out=h0[:, b * K:(b + 1) * K], in0=tmix[:], scalar=1.0,
                    in1=nn[:], op0=ALU.add, op1=ALU.mult)

            if debug:
                nc.sync.dma_start(dbg_h0[:], h0[:])
            # ---------------- head MLP ----------------
            out_row = sg.tile([1, NBLK * P], F32, tag="outrow")
            for b in range(NBLK):
                tps = ps_rt.tile([P, P], F32, tag="rt", name="tps")
                nc.tensor.transpose(tps[:], h0[:, b * K:(b + 1) * K], ident[:])
                hT = sg.tile([P, P], F32, tag="hT")
                nc.scalar.copy(hT[:], tps[:])

                ps1 = ps_rt.tile([P, P], F32, tag="rt", name="ps1")
                nc.tensor.matmul(ps1[:], lhsT=w1h[:], rhs=hT[:],
                                 start=True, stop=True)
                hb1 = sg.tile([P, P], F32, tag="hb1")
                nc.vector.tensor_scalar(out=hb1[:], in0=ps1[:],
                                        scalar1=b1hc[:], scalar2=None,
                                        op0=ALU.add)
                sg1 = sg.tile([P, P], F32, tag="sg1")
                nc.scalar.activation(sg1[:], hb1[:], ACTF.Sigmoid,
                                     bias=constcol(0.0), scale=1.0)
                s1 = sg.tile([P, P], F32, tag="s1")
                nc.vector.tensor_mul(s1[:], sg1[:], hb1[:])

                ps2 = ps_rt.tile([P, P], F32, tag="rt", name="ps2")
                nc.tensor.matmul(ps2[:], lhsT=w2h[:], rhs=s1[:],
                                 start=True, stop=True)
                hb2 = sg.tile([P, P], F32, tag="hb2")
                nc.vector.tensor_scalar(out=hb2[:], in0=ps2[:],
                                        scalar1=b2hc[:], scalar2=None,
                                        op0=ALU.add)
                sg2 = sg.tile([P, P], F32, tag="sg2")
                nc.scalar.activation(sg2[:], hb2[:], ACTF.Sigmoid,
                                     bias=constcol(0.0), scale=1.0)
                s2 = sg.tile([P, P], F32, tag="s2")
                nc.vector.tensor_mul(s2[:], sg2[:], hb2[:])

                ps3 = ps_rt.tile([1, P], F32, tag="rt", name="ps3")
                nc.tensor.matmul(ps3[:], lhsT=wlast[:], rhs=s2[:],
                                 start=True, stop=True)
                nc.scalar.activation(out_row[:, b * P:(b + 1) * P], ps3[:],
                                     ACTF.Identity, bias=lastb[:], scale=1.0)
            nc.sync.dma_start(out_d[:], out_row[:])

    nc.compile()
    return nc, T


def _host_prep(inputs):
    """Sort/pad pairs, build per-core arrays (incl. per-pair embedding
    planes -- a host-side table lookup, like the position planes)."""
    embt = (np.asarray(inputs['embed'], dtype=np.float32)
            * np.float32(INIT_SCALE * MSG_SCALE)).astype(BF16NP)
    pos = np.ascontiguousarray(inputs['positions'], dtype=np.float32)
    cells = np.ascontiguousarray(inputs['cells'], dtype=np.float32)
    species = np.asarray(inputs['species']).astype(np.int64)
    shifts = np.asarray(inputs['cell_shifts']).astype(np.float32)
    ci = np.asarray(inputs['center_indices']).astype(np.int64)
    ni = np.asarray(inputs['neighbor_indices']).astype(np.int64)
    sp = np.asarray(inputs['structure_pairs']).astype(np.int64)

    order = np.argsort(ci, kind='stable')
    ci_s, ni_s, sp_s = ci[order], ni[order], sp[order]
    shifts_s = shifts[order]

    blk_of_pair = ci_s // P          # global 128-block id (0..39)
    nblocks = NCORES * NBLK          # 40 global blocks; core c owns 5c..5c+4
    counts = np.bincount(blk_of_pair, minlength=nblocks)
    TB = int(np.max((counts + P - 1) // P))
    T = NBLK * TB
    PP = T * P
    TBP = TB * P
    starts = np.zeros(nblocks + 1, np.int64)
    np.cumsum(counts, out=starts[1:])

    cores = []
    for c in range(NCORES):
        # slot -> original sorted-pair index, -1 for dummy
        slot_src = np.full(PP, -1, np.int64)
        lc = np.full(PP, -1.0, np.float32)
        for b in range(NBLK):
            g = c * NBLK + b
            cnt = counts[g]
            s0 = b * TBP
            slot_src[s0:s0 + cnt] = np.arange(starts[g], starts[g] + cnt)
            lc[s0:s0 + cnt] = (ci_s[starts[g]:starts[g] + cnt]
                               - g * P).astype(np.float32)
        real = slot_src >= 0
        src = np.where(real, slot_src, 0)

        nbr = np.where(real, ni_s[src], 0).astype(np.int32)
        spc = np.where(real, species[ni_s[src]], 0).astype(np.int64)

        pc = np.where(real[:, None], pos[ci_s[src]], 0.0).astype(np.float32)
        pn = pos[ni_s[src]].astype(np.float32).copy()
        pn[~real] = 0.0
        pn[~real, 0] = 10.0          # dummy d = 10 (outside cutoff)
        sh = np.where(real[:, None], shifts_s[src], 1.0).astype(np.float32)
        cl = np.where(real[:, None, None], cells[sp_s[src]], 0.0)
        cl = cl.reshape(PP, 9).astype(np.float32)

        def plane(v):   # slot r = b*TB*P + q*TB + t  ->  [q, b*TB + t]
            return np.ascontiguousarray(
                v.reshape(NBLK, P, TB).transpose(1, 0, 2).reshape(P, T))

        def seg(v):     # slot r = 128*i + q  ->  [q, i]
            return np.ascontiguousarray(v.reshape(T, P).T)

        m = {}
        pl = ([plane(pc[:, j]) for j in range(3)]
              + [plane(pn[:, j]) for j in range(3)]
              + [plane(sh[:, j]) for j in range(3)]
              + [plane(cl[:, j]) for j in range(9)])
        m['planes'] = np.ascontiguousarray(np.stack(pl, axis=0))
        lcs = seg(lc)                     # [P, T] local centers, -1 dummies
        sel = (lcs[:, :, None] == np.arange(P, dtype=np.float32)[None, None, :])
        m['sel'] = np.ascontiguousarray(
            sel.reshape(P, T * P).astype(BF16NP))
        m['ones'] = np.ones((1, TBP), BF16NP)

        def wrap16(v):  # slot i of block b -> [i % 16, b*TB*8 + i//16],
            a = v.reshape(NBLK, TBP // 16, 16)    # replicated across the
            a = a.transpose(0, 2, 1)              # eight 16-partition groups
            a = np.tile(a, (1, 8, 1))
            return np.ascontiguousarray(
                a.transpose(1, 0, 2).reshape(P, NBLK * (TBP // 16))
            ).astype(np.int16)

        g_blk, g_r = nbr // P, nbr % P
        nbr_slab = ((g_blk % NBLK) * NCORES + g_blk // NBLK) * P + g_r
        m['nbr16'] = wrap16(nbr_slab.astype(np.int64))
        emb_n = embt[spc]                     # [PP, K] bf16
        m['embp'] = np.ascontiguousarray(
            emb_n.reshape(T, P, K).transpose(1, 0, 2).reshape(P, T * K))
        cores.append(m)
    return cores, TB


def _make_weights(inputs):
    f32 = lambda k: np.asarray(inputs[k], dtype=np.float32)
    MS2 = np.float32((1.0 + INIT_SCALE) * MSG_SCALE)
    w1cat9 = np.zeros((NB + 1, 2 * NH), np.float32)
    w1cat9[:NB, :NH] = f32('rad_w1')
    w1cat9[:NB, NH:] = f32('erad_w1')
    w1cat9[NB, :NH] = f32('rad_b1')
    w1cat9[NB, NH:] = f32('erad_b1')
    # rt = cat.T @ w2e : cols 0:K = Rk0(rad), K:2K = MS2*Rke0(erad)
    w2e = np.zeros((CAT, 2 * K), np.float32)
    w2e[:NH, :K] = f32('rad_w2')[:, :K]
    w2e[NH:2 * NH, K:2 * K] = f32('erad_w2')[:, :K] * MS2
    w2e[2 * NH, :K] = f32('rad_b2')[:K]
    w2e[2 * NH, K:2 * K] = f32('erad_b2')[:K] * MS2

    mix0m = np.ascontiguousarray(
        np.broadcast_to(f32('mix_a')[0][None, :], (P, K)))
    emix0m = np.ascontiguousarray(
        np.broadcast_to(f32('emix_a')[0][None, :], (P, K)))

    return {
        'w1cat9': w1cat9.astype(BF16NP), 'w2e': w2e.astype(BF16NP),
        'mix0m': mix0m, 'emix0m': emix0m,
        'w1h': f32('head_w1'), 'w2h': f32('head_w2'),
        'wlast': np.ascontiguousarray(f32('last_w').reshape(K, 1)),
        'b1hc': np.ascontiguousarray(f32('head_b1').reshape(K, 1)),
        'b2hc': np.ascontiguousarray(f32('head_b2').reshape(K, 1)),
        'lastb': np.ascontiguousarray(f32('last_b').reshape(1, 1)),
    }


def kernel(**inputs):
    weights = _make_weights(inputs)
    cores, TB = _host_prep(inputs)
    if TB not in _prog_cache:
        _prog_cache[TB] = _build_program(TB)
    nc, T = _prog_cache[TB]

    in_maps = [{**weights, **cores[c]} for c in range(NCORES)]
    res = run_bass_kernel_spmd(nc, in_maps, list(range(NCORES)))
    global _last_results
    _last_results = res
    out = np.concatenate(
        [res.results[c]['out'].reshape(-1) for c in range(NCORES)])
    return out[:N_ATOMS].reshape(N_ATOMS, 1).astype(np.float32)
